# revision 3
# baseline (speedup 1.0000x reference)
"""Multi-head attention (B=2, N=2048, C=768, H=12) on 8 Trainium2 NeuronCores.

Sharding: core c handles batch b=c//4 and head-group g=c%4 (3 heads, 192 dims).
Host side compacts rows where mask==0 out of x (they only matter via the
uniform-attention fallback, which reduces to a single mean-value row),
pre-transposes weight slices, and casts matmul operands to bf16.

Device per core:
  q_T/k_T = WqT.T @ xcT, v = xcT.T @ WvT           (bf16 matmuls, fp32 psum)
  scores_T[k, q] = k_T.T-slice @ q_T                (keys on partitions)
  attn_T = exp(0.125*scores + bias[key])            (bias=-1000 marks pad keys)
  outT[65, q] = sum_kt v_aug[kt].T @ attn_T[kt]     (row 64 = softmax sums)
  out_norm = outT[:64] * bcast(1/sums)              (bcast via K=1 matmul)
  proj[q, 768] = sum_h out_norm_h.T @ WoT_h + bo/4  (bias via K=1 matmul)
  ReduceScatter(add) over the 4 cores of the same batch.

Host reassembles kept rows and fills masked rows with the reduced
mean-value projection row.
"""

import functools
import numpy as np
import ml_dtypes

import concourse.tile as tile
import concourse.mybir as mybir
from concourse import bacc
from concourse.bass_utils import run_bass_kernel_spmd

B, N, C = 2, 2048, 768
H, D = 12, 64
NCORES, NGROUPS, HPG = 8, 4, 3     # 4 head-groups of 3 heads; 2 batches
HD = HPG * D                       # 192 head dims per core
SCALE = float(D) ** -0.5           # 0.125
CT = C // 128                      # 6 contraction tiles of 128
BF16 = mybir.dt.bfloat16
F32 = mybir.dt.float32
NPBF16 = ml_dtypes.bfloat16


def _qchunks(kp, cnt, width):
    """[(start, size)] covering the real (un-padded) query rows [0, cnt)
    in blocks of `width`."""
    out = []
    s = 0
    while s < cnt:
        out.append((s, min(width, cnt - s)))
        s += width
    return out


@functools.lru_cache(maxsize=4)
def _build(kp, cnt_max, reps=1, with_rs=True):
    """Build + compile the SPMD program for padded kept-count `kp`."""
    kt_n = kp // 128
    nc = bacc.Bacc("TRN2", target_bir_lowering=False, debug=False,
                   num_devices=NCORES)

    xcT = nc.dram_tensor("xcT", [C, kp], BF16, kind="ExternalInput").ap()
    kcb = nc.dram_tensor("kcb", [128, kt_n], F32, kind="ExternalInput").ap()
    wqT = nc.dram_tensor("wqT", [C, HD], BF16, kind="ExternalInput").ap()
    wkT = nc.dram_tensor("wkT", [C, HD], BF16, kind="ExternalInput").ap()
    wvT = nc.dram_tensor("wvT", [C, HD], BF16, kind="ExternalInput").ap()
    woT = nc.dram_tensor("woT", [HD, C], BF16, kind="ExternalInput").ap()
    mxT = nc.dram_tensor("mxT", [C, 1], BF16, kind="ExternalInput").ap()
    bo4 = nc.dram_tensor("bo4", [1, C], BF16, kind="ExternalInput").ap()

    kpb = kp + 4                   # + mvproj row + 3 pad rows, divisible by 4
    out_rows = kpb // 4
    out_ext = nc.dram_tensor("out", [out_rows, C], F32,
                             kind="ExternalOutput").ap()

    with tile.TileContext(nc) as tc:
        for _ in range(reps):
            _emit(tc, nc, kp, cnt_max, kt_n, kpb,
                  xcT, kcb, wqT, wkT, wvT, woT, mxT, bo4, out_ext,
                  with_rs=with_rs)
    nc.compile()
    return nc


def _emit(tc, nc, kp, cnt, kt_n, kpb,
          xcT, kcb, wqT, wkT, wvT, woT, mxT, bo4, out_ext, with_rs=True):
    with tc.tile_pool(name="const", bufs=1) as consts, \
         tc.tile_pool(name="dram", bufs=1, space="DRAM") as dram:

        # ---- static loads ------------------------------------------------
        wk_sb = consts.tile([128, CT, HD], BF16)
        nc.sync.dma_start(wk_sb[:], wkT.rearrange("(t p) d -> p t d", p=128))
        wv_sb = consts.tile([128, CT, HD], BF16)
        nc.sync.dma_start(wv_sb[:], wvT.rearrange("(t p) d -> p t d", p=128))
        xcT_t = xcT.rearrange("(t p) n -> t p n", p=128)
        xq = consts.tile([128, CT, kp], BF16)          # x compact, transposed
        for ct in range(CT):
            nc.sync.dma_start(xq[:, ct, :], xcT_t[ct])
        wq_sb = consts.tile([128, CT, HD], BF16)
        nc.sync.dma_start(wq_sb[:], wqT.rearrange("(t p) d -> p t d", p=128))
        kcb_sb = consts.tile([128, kt_n], F32)         # exp bias per key
        nc.sync.dma_start(kcb_sb[:], kcb[:])
        wo_sb = consts.tile([128, 2, C], BF16)         # rows 0..127 | 128..191
        nc.sync.dma_start(wo_sb[:, 0, :], woT[0:128, :])
        nc.sync.dma_start(wo_sb[0:64, 1, :], woT[128:HD, :])
        mx_sb = consts.tile([128, CT, 1], BF16)
        nc.sync.dma_start(mx_sb[:], mxT.rearrange("(t p) o -> p t o", p=128))
        bo_sb = consts.tile([1, C], BF16)
        nc.sync.dma_start(bo_sb[:], bo4[:])
        ones128 = consts.tile([1, 128], BF16)
        nc.vector.memset(ones128[:], 1.0)
        ones64 = consts.tile([1, 64], BF16)
        nc.vector.memset(ones64[:], 1.0)

        rs_in = dram.tile([kpb, C], F32)
        rs_out = dram.tile([kpb // 4, C], F32)

        # ---- QKV projections --------------------------------------------
        q_pair = consts.tile([128, kp], BF16, tag="q_pair")   # heads 0,1
        q_solo = consts.tile([64, kp], BF16, tag="q_solo")    # head 2
        k_pair = consts.tile([128, kp], BF16, tag="k_pair")
        k_solo = consts.tile([64, kp], BF16, tag="k_solo")
        v_aug = consts.tile([128, kt_n, HPG, D + 1], BF16, tag="v_aug")
        nc.vector.memset(v_aug[:, :, :, D:D + 1], 1.0)        # sums column

        with tc.tile_pool(name="qkv_ps", bufs=4, space="PSUM") as pps:
            nb = 1024
            for (w_sb, pair, solo) in ((wk_sb, k_pair, k_solo),
                                       (wq_sb, q_pair, q_solo)):
                for s in range(0, kp, nb):
                    w = min(nb, kp - s)
                    ps = pps.tile([128, nb], F32, tag="qk")
                    for ct in range(CT):
                        for ms in range(0, w, 512):
                            mw = min(512, w - ms)
                            nc.tensor.matmul(ps[:, ms:ms + mw],
                                             w_sb[:, ct, 0:128],
                                             xq[:, ct, s + ms:s + ms + mw],
                                             start=(ct == 0), stop=(ct == CT - 1))
                    nc.scalar.copy(pair[:, s:s + w], ps[:, :w])
                    ps2 = pps.tile([64, nb], F32, tag="qk")
                    for ct in range(CT):
                        for ms in range(0, w, 512):
                            mw = min(512, w - ms)
                            nc.tensor.matmul(ps2[:64, ms:ms + mw],
                                             w_sb[:, ct, 128:HD],
                                             xq[:, ct, s + ms:s + ms + mw],
                                             start=(ct == 0), stop=(ct == CT - 1))
                    nc.scalar.copy(solo[:, s:s + w], ps2[:64, :w])
            for kt in range(kt_n):
                ps = pps.tile([128, HD], F32, tag="qk")
                for ct in range(CT):
                    nc.tensor.matmul(ps[:, 0:HD],
                                     xq[:, ct, kt * 128:(kt + 1) * 128],
                                     wv_sb[:, ct, :],
                                     start=(ct == 0), stop=(ct == CT - 1))
                nc.vector.tensor_copy(
                    v_aug[:, kt, :, 0:D],
                    ps[:, 0:HD].rearrange("p (h d) -> p h d", h=HPG))

            # ---- mean-v row (for fully-masked queries) -------------------
            mv_ps = pps.tile([128, 1], F32, tag="qk")
            for ct in range(CT):
                nc.tensor.matmul(mv_ps[:], wv_sb[:, ct, 0:128],
                                 mx_sb[:, ct, :],
                                 start=(ct == 0), stop=(ct == CT - 1))
            mv_ps2 = pps.tile([64, 1], F32, tag="qk")
            for ct in range(CT):
                nc.tensor.matmul(mv_ps2[:], wv_sb[:, ct, 128:HD],
                                 mx_sb[:, ct, :],
                                 start=(ct == 0), stop=(ct == CT - 1))
            mvT = consts.tile([128, 1], BF16, tag="mvT")
            nc.vector.tensor_copy(mvT[:], mv_ps[:])
            mvT2 = consts.tile([64, 1], BF16, tag="mvT2")
            nc.vector.tensor_copy(mvT2[:], mv_ps2[:])
            mvproj_sb = consts.tile([1, C], F32, tag="mvp_sb")
            mp = pps.tile([1, C], F32, tag="qk")
            for cs in range(0, C, 512):
                cw = min(512, C - cs)
                nc.tensor.matmul(mp[:, cs:cs + cw], mvT[:],
                                 wo_sb[:, 0, cs:cs + cw],
                                 start=True, stop=False)
                nc.tensor.matmul(mp[:, cs:cs + cw], mvT2[:],
                                 wo_sb[0:64, 1, cs:cs + cw],
                                 start=False, stop=False)
                nc.tensor.matmul(mp[:, cs:cs + cw], ones128[:, 0:1],
                                 bo_sb[:, cs:cs + cw],
                                 start=False, stop=True)
            nc.vector.tensor_copy(mvproj_sb[:], mp[:])
            nc.sync.dma_start(rs_in[kp:kp + 1, :], mvproj_sb[:])

        # ---- attention + projection, span-major --------------------------
        # First span ~3/4 of the queries: the smaller second span's attention
        # still hides the first span's projection + output DMA.
        if cnt > 1024:
            span_w = max(512, min(1024, (7 * cnt // 8) // 128 * 128))
        else:
            span_w = min(cnt, 1024)

        with tc.tile_pool(name="att_ps", bufs=2, space="PSUM") as aps, \
             tc.tile_pool(name="att_sb", bufs=3) as asb, \
             tc.tile_pool(name="on_sb", bufs=2) as onsb, \
             tc.tile_pool(name="o_ps", bufs=1, space="PSUM") as ops, \
             tc.tile_pool(name="pj_ps", bufs=2, space="PSUM") as jps, \
             tc.tile_pool(name="pj_sb", bufs=4) as jsb:
            for (qs, qw) in _qchunks(kp, cnt, span_w):
                on_pair = onsb.tile([128, span_w], BF16, tag="on_pair")
                on_solo = onsb.tile([64, span_w], BF16, tag="on_solo")
                for h in range(HPG):
                    if h < 2:
                        k_src, k_lo = k_pair, 64 * h
                        q_src, q_lo = q_pair, 64 * h
                        on_dst, on_lo = on_pair, 64 * h
                    else:
                        k_src, k_lo = k_solo, 0
                        q_src, q_lo = q_solo, 0
                        on_dst, on_lo = on_solo, 0
                    o_ps = ops.tile([D + 1, span_w], F32, tag="o")
                    for kt in range(kt_n):
                        s_ps = aps.tile([128, span_w], F32, tag="s")
                        for ms in range(0, qw, 512):
                            mw = min(512, qw - ms)
                            nc.tensor.matmul(
                                s_ps[:, ms:ms + mw],
                                k_src[k_lo:k_lo + D, kt * 128:(kt + 1) * 128],
                                q_src[q_lo:q_lo + D, qs + ms:qs + ms + mw],
                                start=True, stop=True)
                        attn = asb.tile([128, span_w], BF16, tag="attn")
                        nc.scalar.activation(attn[:, :qw], s_ps[:, :qw],
                                             mybir.ActivationFunctionType.Exp,
                                             bias=kcb_sb[:, kt:kt + 1],
                                             scale=SCALE)
                        for ms in range(0, qw, 512):
                            mw = min(512, qw - ms)
                            nc.tensor.matmul(
                                o_ps[:, ms:ms + mw],
                                v_aug[:, kt, h, :],
                                attn[:, ms:ms + mw],
                                start=(kt == 0), stop=(kt == kt_n - 1))
                    # normalize: out / sums, with 1/sums broadcast down the
                    # partitions (all on DVE; frees o_ps after one copy)
                    o_c = asb.tile([D + 1, span_w], F32, tag="o_c")
                    nc.vector.tensor_copy(o_c[:, :qw], o_ps[:, :qw])
                    rec = asb.tile([1, span_w], F32, tag="rec")
                    nc.vector.reciprocal(rec[:, :qw], o_ps[D:D + 1, :qw])
                    rec_bc = asb.tile([D, span_w], F32, tag="rec_bc")
                    nc.gpsimd.partition_broadcast(rec_bc[:, :qw], rec[:, :qw])
                    nc.vector.tensor_mul(on_dst[on_lo:on_lo + D, :qw],
                                         o_c[0:D, :qw], rec_bc[:, :qw])
                # output projection + bias for this span's query chunks,
                # pipelined through two 1-bank psum tiles (512-wide halves)
                for qc in range(0, qw, 128):
                    cw = min(128, qw - qc)
                    for ci, cs in enumerate(range(0, C, 512)):
                        ccw = min(512, C - cs)
                        pj = jps.tile([128, 512], F32, tag="pj")
                        nc.tensor.matmul(pj[:cw, :ccw],
                                         on_pair[:, qc:qc + cw],
                                         wo_sb[:, 0, cs:cs + ccw],
                                         start=True, stop=False)
                        nc.tensor.matmul(pj[:cw, :ccw],
                                         on_solo[:, qc:qc + cw],
                                         wo_sb[0:64, 1, cs:cs + ccw],
                                         start=False, stop=False)
                        nc.tensor.matmul(pj[:cw, :ccw],
                                         ones128[:, 0:cw], bo_sb[:, cs:cs + ccw],
                                         start=False, stop=True)
                        pj_sb = jsb.tile([128, 512], F32, tag="pj_sb")
                        if ci % 2 == 0:
                            nc.vector.tensor_copy(pj_sb[:cw, :ccw],
                                                  pj[:cw, :ccw])
                        else:
                            nc.scalar.copy(pj_sb[:cw, :ccw], pj[:cw, :ccw])
                        nc.sync.dma_start(
                            rs_in[qs + qc:qs + qc + cw, cs:cs + ccw],
                            pj_sb[:cw, :ccw])

        # ---- reduce-scatter over the 4 cores of this batch ---------------
        if with_rs:
            nc.gpsimd.collective_compute(
                "ReduceScatter", mybir.AluOpType.add,
                replica_groups=[[0, 1, 2, 3], [4, 5, 6, 7]],
                ins=[rs_in[:]], outs=[rs_out[:]])
            nc.sync.dma_start(out_ext[:], rs_out[:])
        else:
            nc.sync.dma_start(out_ext[:], rs_in[0:kpb // 4, :])


def make_in_maps(inputs, kept, cnt, kp):
    x = np.asarray(inputs["x"], dtype=np.float32)
    Wq, Wk, Wv, Wo = (np.asarray(inputs[k], np.float32)
                      for k in ("Wq", "Wk", "Wv", "Wo"))
    bo = np.asarray(inputs["bo"], np.float32)
    woT_full = np.ascontiguousarray(Wo.T)          # [hd_in, c_out]
    in_maps = []
    for c in range(NCORES):
        b, g = divmod(c, NGROUPS)
        hs = slice(g * HD, (g + 1) * HD)
        xc = np.zeros((kp, C), np.float32)
        xc[:cnt[b]] = x[b][kept[b]]
        kcb_flat = np.full(kp, -1000.0, np.float32)
        kcb_flat[:cnt[b]] = 0.0
        kcb = np.ascontiguousarray(kcb_flat.reshape(kp // 128, 128).T)
        in_maps.append({
            "xcT": np.ascontiguousarray(xc.T).astype(NPBF16),
            "kcb": kcb,
            "wqT": np.ascontiguousarray(Wq[hs].T).astype(NPBF16),
            "wkT": np.ascontiguousarray(Wk[hs].T).astype(NPBF16),
            "wvT": np.ascontiguousarray(Wv[hs].T).astype(NPBF16),
            "woT": np.ascontiguousarray(woT_full[hs]).astype(NPBF16),
            "mxT": x[b].mean(0).reshape(C, 1).astype(NPBF16),
            "bo4": (bo / NGROUPS).reshape(1, C).astype(NPBF16),
        })
    return in_maps


def kernel(x, mask, Wq, Wk, Wv, Wo, bo):
    x = np.asarray(x, dtype=np.float32)
    mask = np.asarray(mask)
    kept = [np.nonzero(mask[b])[0] for b in range(B)]
    cnt = [len(k) for k in kept]
    cnt_max = max(max(cnt), 1)
    kp = max(128, -(-cnt_max // 128) * 128)

    nc = _build(kp, cnt_max)
    in_maps = make_in_maps(
        {"x": x, "Wq": Wq, "Wk": Wk, "Wv": Wv, "Wo": Wo, "bo": bo},
        kept, cnt, kp)

    r = run_bass_kernel_spmd(nc, in_maps, core_ids=list(range(NCORES)))
    global LAST_HW_NS, LAST_RESULT
    LAST_RESULT = r
    if getattr(r, "exec_time_ns", None):
        LAST_HW_NS = r.exec_time_ns

    out = np.empty((B, N, C), np.float32)
    for b in range(B):
        rs = np.concatenate([r.results[NGROUPS * b + i]["out"]
                             for i in range(NGROUPS)], axis=0)
        out[b, kept[b]] = rs[:cnt[b]]
        out[b, mask[b] == 0] = rs[kp]
    return out



# revision 12
# speedup vs baseline: 1.2786x; 1.2786x over previous
"""Multi-head attention (B=2, N=2048, C=768, H=12) on 8 Trainium2 NeuronCores.

Sharding: core c handles batch b=c//4 and head-group g=c%4 (3 heads, 192 dims).
Host side compacts rows where mask==0 out of x (they only matter via the
uniform-attention fallback, which reduces to a single mean-value row),
pre-transposes weight slices, and casts matmul operands to bf16.

Device per core:
  q_T/k_T = WqT.T @ xcT, v = xcT.T @ WvT           (bf16 matmuls, fp32 psum)
  scores_T[k, q] = k_T.T-slice @ q_T                (keys on partitions)
  attn_T = exp(0.125*scores + bias[key])            (bias=-1000 marks pad keys)
  outT[65, q] = sum_kt v_aug[kt].T @ attn_T[kt]     (row 64 = softmax sums)
  out_norm = outT[:64] * bcast(1/sums)              (bcast via K=1 matmul)
  proj[q, 768] = sum_h out_norm_h.T @ WoT_h + bo/4  (bias via K=1 matmul)
  ReduceScatter(add) over the 4 cores of the same batch.

Host reassembles kept rows and fills masked rows with the reduced
mean-value projection row.
"""

import functools
import numpy as np
import ml_dtypes

import concourse.tile as tile
import concourse.mybir as mybir
from concourse import bacc
from concourse.bass_utils import run_bass_kernel_spmd

B, N, C = 2, 2048, 768
H, D = 12, 64
NCORES, NGROUPS, HPG = 8, 4, 3     # 4 head-groups of 3 heads; 2 batches
HD = HPG * D                       # 192 head dims per core
SCALE = float(D) ** -0.5           # 0.125
CT = C // 128                      # 6 contraction tiles of 128
BF16 = mybir.dt.bfloat16
F32 = mybir.dt.float32
NPBF16 = ml_dtypes.bfloat16


def _qchunks(kp, cnt, width):
    """[(start, size)] covering the real (un-padded) query rows [0, cnt)
    in blocks of `width`."""
    out = []
    s = 0
    while s < cnt:
        out.append((s, min(width, cnt - s)))
        s += width
    return out


@functools.lru_cache(maxsize=4)
def _build(kp, cnt_max, reps=1, with_rs=True, rs_dt=F32):
    """Build + compile the SPMD program for padded kept-count `kp`."""
    kt_n = kp // 128
    nc = bacc.Bacc("TRN2", target_bir_lowering=False, debug=False,
                   num_devices=NCORES)

    xcT = nc.dram_tensor("xcT", [C, kp], BF16, kind="ExternalInput").ap()
    kcb = nc.dram_tensor("kcb", [128, kt_n], F32, kind="ExternalInput").ap()
    wqT = nc.dram_tensor("wqT", [C, HD], BF16, kind="ExternalInput").ap()
    wkT = nc.dram_tensor("wkT", [C, HD], BF16, kind="ExternalInput").ap()
    wvT = nc.dram_tensor("wvT", [C, HD], BF16, kind="ExternalInput").ap()
    woT = nc.dram_tensor("woT", [HD, C], BF16, kind="ExternalInput").ap()
    mxT = nc.dram_tensor("mxT", [C, 1], BF16, kind="ExternalInput").ap()
    bo4 = nc.dram_tensor("bo4", [1, C], BF16, kind="ExternalInput").ap()

    kpb = kp + 4                   # + mvproj row + 3 pad rows, divisible by 4
    out_rows = kpb // 4
    out_ext = nc.dram_tensor("out", [out_rows, C], rs_dt,
                             kind="ExternalOutput").ap()

    with tile.TileContext(nc) as tc:
        for _ in range(reps):
            _emit(tc, nc, kp, cnt_max, kt_n, kpb,
                  xcT, kcb, wqT, wkT, wvT, woT, mxT, bo4, out_ext,
                  with_rs=with_rs, rs_dt=rs_dt)
    nc.compile()
    return nc


def _emit(tc, nc, kp, cnt, kt_n, kpb,
          xcT, kcb, wqT, wkT, wvT, woT, mxT, bo4, out_ext, with_rs=True,
          rs_dt=F32):
    with tc.tile_pool(name="const", bufs=1) as consts, \
         tc.tile_pool(name="dram", bufs=1, space="DRAM") as dram:

        # ---- static loads ------------------------------------------------
        wk_sb = consts.tile([128, CT, HD], BF16)
        nc.sync.dma_start(wk_sb[:], wkT.rearrange("(t p) d -> p t d", p=128))
        wv_sb = consts.tile([128, CT, HD], BF16)
        nc.sync.dma_start(wv_sb[:], wvT.rearrange("(t p) d -> p t d", p=128))
        xcT_t = xcT.rearrange("(t p) n -> t p n", p=128)
        xq = consts.tile([128, CT, kp], BF16)          # x compact, transposed
        for ct in range(CT):
            nc.sync.dma_start(xq[:, ct, :], xcT_t[ct])
        wq_sb = consts.tile([128, CT, HD], BF16)
        nc.sync.dma_start(wq_sb[:], wqT.rearrange("(t p) d -> p t d", p=128))
        kcb_sb = consts.tile([128, kt_n], F32)         # exp bias per key
        nc.sync.dma_start(kcb_sb[:], kcb[:])
        wo_sb = consts.tile([128, 2, C], BF16)         # rows 0..127 | 128..191
        nc.sync.dma_start(wo_sb[:, 0, :], woT[0:128, :])
        nc.sync.dma_start(wo_sb[0:64, 1, :], woT[128:HD, :])
        mx_sb = consts.tile([128, CT, 1], BF16)
        nc.sync.dma_start(mx_sb[:], mxT.rearrange("(t p) o -> p t o", p=128))
        bo_sb = consts.tile([1, C], BF16)
        nc.sync.dma_start(bo_sb[:], bo4[:])
        ones128 = consts.tile([1, 128], BF16)
        nc.vector.memset(ones128[:], 1.0)
        ones64 = consts.tile([1, 64], BF16)
        nc.vector.memset(ones64[:], 1.0)

        rs_in = dram.tile([kpb, C], rs_dt)
        rs_out = dram.tile([kpb // 4, C], rs_dt)

        # ---- QKV projections --------------------------------------------
        q_pair = consts.tile([128, kp], BF16, tag="q_pair")   # heads 0,1
        q_solo = consts.tile([64, kp], BF16, tag="q_solo")    # head 2
        k_pair = consts.tile([128, kp], BF16, tag="k_pair")
        k_solo = consts.tile([64, kp], BF16, tag="k_solo")
        v_aug = consts.tile([128, kt_n, HPG, D + 1], BF16, tag="v_aug")
        nc.vector.memset(v_aug[:, :, :, D:D + 1], 1.0)        # sums column

        with tc.tile_pool(name="qkv_ps", bufs=4, space="PSUM") as pps:
            nb = 1024
            for (w_sb, pair, solo) in ((wk_sb, k_pair, k_solo),
                                       (wq_sb, q_pair, q_solo)):
                for s in range(0, kp, nb):
                    w = min(nb, kp - s)
                    ps = pps.tile([128, nb], F32, tag="qk")
                    for ct in range(CT):
                        for ms in range(0, w, 512):
                            mw = min(512, w - ms)
                            nc.tensor.matmul(ps[:, ms:ms + mw],
                                             w_sb[:, ct, 0:128],
                                             xq[:, ct, s + ms:s + ms + mw],
                                             start=(ct == 0), stop=(ct == CT - 1))
                    nc.scalar.copy(pair[:, s:s + w], ps[:, :w])
                    ps2 = pps.tile([64, nb], F32, tag="qk")
                    for ct in range(CT):
                        for ms in range(0, w, 512):
                            mw = min(512, w - ms)
                            nc.tensor.matmul(ps2[:64, ms:ms + mw],
                                             w_sb[:, ct, 128:HD],
                                             xq[:, ct, s + ms:s + ms + mw],
                                             start=(ct == 0), stop=(ct == CT - 1))
                    nc.scalar.copy(solo[:, s:s + w], ps2[:64, :w])
            for kt in range(kt_n):
                ps = pps.tile([128, HD], F32, tag="qk")
                for ct in range(CT):
                    nc.tensor.matmul(ps[:, 0:HD],
                                     xq[:, ct, kt * 128:(kt + 1) * 128],
                                     wv_sb[:, ct, :],
                                     start=(ct == 0), stop=(ct == CT - 1))
                nc.vector.tensor_copy(
                    v_aug[:, kt, :, 0:D],
                    ps[:, 0:HD].rearrange("p (h d) -> p h d", h=HPG))

            # ---- mean-v row (for fully-masked queries) -------------------
            mv_ps = pps.tile([128, 1], F32, tag="qk")
            for ct in range(CT):
                nc.tensor.matmul(mv_ps[:], wv_sb[:, ct, 0:128],
                                 mx_sb[:, ct, :],
                                 start=(ct == 0), stop=(ct == CT - 1))
            mv_ps2 = pps.tile([64, 1], F32, tag="qk")
            for ct in range(CT):
                nc.tensor.matmul(mv_ps2[:], wv_sb[:, ct, 128:HD],
                                 mx_sb[:, ct, :],
                                 start=(ct == 0), stop=(ct == CT - 1))
            mvT = consts.tile([128, 1], BF16, tag="mvT")
            nc.vector.tensor_copy(mvT[:], mv_ps[:])
            mvT2 = consts.tile([64, 1], BF16, tag="mvT2")
            nc.vector.tensor_copy(mvT2[:], mv_ps2[:])
            mvproj_sb = consts.tile([1, C], rs_dt, tag="mvp_sb")
            mp = pps.tile([1, C], F32, tag="qk")
            for cs in range(0, C, 512):
                cw = min(512, C - cs)
                nc.tensor.matmul(mp[:, cs:cs + cw], mvT[:],
                                 wo_sb[:, 0, cs:cs + cw],
                                 start=True, stop=False)
                nc.tensor.matmul(mp[:, cs:cs + cw], mvT2[:],
                                 wo_sb[0:64, 1, cs:cs + cw],
                                 start=False, stop=False)
                nc.tensor.matmul(mp[:, cs:cs + cw], ones128[:, 0:1],
                                 bo_sb[:, cs:cs + cw],
                                 start=False, stop=True)
            nc.vector.tensor_copy(mvproj_sb[:], mp[:])
            nc.sync.dma_start(rs_in[kp:kp + 1, :], mvproj_sb[:])

        # ---- attention + projection, span-major --------------------------
        # First span ~3/4 of the queries: the smaller second span's attention
        # still hides the first span's projection + output DMA.
        if cnt > 1024:
            span_w = max(512, min(1024, (7 * cnt // 8) // 128 * 128))
        else:
            span_w = min(cnt, 1024)

        with tc.tile_pool(name="att_ps", bufs=2, space="PSUM") as aps, \
             tc.tile_pool(name="att_sb", bufs=3) as asb, \
             tc.tile_pool(name="on_sb", bufs=2) as onsb, \
             tc.tile_pool(name="o_ps", bufs=1, space="PSUM") as ops, \
             tc.tile_pool(name="pj_ps", bufs=2, space="PSUM") as jps, \
             tc.tile_pool(name="pj_sb", bufs=4) as jsb:
            for (qs, qw) in _qchunks(kp, cnt, span_w):
                on_pair = onsb.tile([128, span_w], BF16, tag="on_pair")
                on_solo = onsb.tile([64, span_w], BF16, tag="on_solo")
                for h in range(HPG):
                    if h < 2:
                        k_src, k_lo = k_pair, 64 * h
                        q_src, q_lo = q_pair, 64 * h
                        on_dst, on_lo = on_pair, 64 * h
                    else:
                        k_src, k_lo = k_solo, 0
                        q_src, q_lo = q_solo, 0
                        on_dst, on_lo = on_solo, 0
                    o_ps = ops.tile([D + 1, span_w], F32, tag="o")
                    for kt in range(kt_n):
                        s_ps = aps.tile([128, span_w], F32, tag="s")
                        for ms in range(0, qw, 512):
                            mw = min(512, qw - ms)
                            nc.tensor.matmul(
                                s_ps[:, ms:ms + mw],
                                k_src[k_lo:k_lo + D, kt * 128:(kt + 1) * 128],
                                q_src[q_lo:q_lo + D, qs + ms:qs + ms + mw],
                                start=True, stop=True)
                        attn = asb.tile([128, span_w], BF16, tag="attn")
                        nc.scalar.activation(attn[:, :qw], s_ps[:, :qw],
                                             mybir.ActivationFunctionType.Exp,
                                             bias=kcb_sb[:, kt:kt + 1],
                                             scale=SCALE)
                        for ms in range(0, qw, 512):
                            mw = min(512, qw - ms)
                            nc.tensor.matmul(
                                o_ps[:, ms:ms + mw],
                                v_aug[:, kt, h, :],
                                attn[:, ms:ms + mw],
                                start=(kt == 0), stop=(kt == kt_n - 1))
                    # normalize: out / sums, with 1/sums broadcast down the
                    # partitions (all on DVE; frees o_ps after one copy)
                    o_c = asb.tile([D + 1, span_w], F32, tag="o_c")
                    nc.vector.tensor_copy(o_c[:, :qw], o_ps[:, :qw])
                    rec = asb.tile([1, span_w], F32, tag="rec")
                    nc.vector.reciprocal(rec[:, :qw], o_ps[D:D + 1, :qw])
                    rec_bc = asb.tile([D, span_w], F32, tag="rec_bc")
                    nc.gpsimd.partition_broadcast(rec_bc[:, :qw], rec[:, :qw])
                    nc.vector.tensor_mul(on_dst[on_lo:on_lo + D, :qw],
                                         o_c[0:D, :qw], rec_bc[:, :qw])
                # output projection + bias for this span's query chunks,
                # pipelined through two 1-bank psum tiles (512-wide halves)
                for qc in range(0, qw, 128):
                    cw = min(128, qw - qc)
                    for ci, cs in enumerate(range(0, C, 512)):
                        ccw = min(512, C - cs)
                        pj = jps.tile([128, 512], F32, tag="pj")
                        nc.tensor.matmul(pj[:cw, :ccw],
                                         on_pair[:, qc:qc + cw],
                                         wo_sb[:, 0, cs:cs + ccw],
                                         start=True, stop=False)
                        nc.tensor.matmul(pj[:cw, :ccw],
                                         on_solo[:, qc:qc + cw],
                                         wo_sb[0:64, 1, cs:cs + ccw],
                                         start=False, stop=False)
                        nc.tensor.matmul(pj[:cw, :ccw],
                                         ones128[:, 0:cw], bo_sb[:, cs:cs + ccw],
                                         start=False, stop=True)
                        pj_sb = jsb.tile([128, 512], rs_dt, tag="pj_sb")
                        if ci % 2 == 0:
                            nc.vector.tensor_copy(pj_sb[:cw, :ccw],
                                                  pj[:cw, :ccw])
                        else:
                            nc.scalar.copy(pj_sb[:cw, :ccw], pj[:cw, :ccw])
                        nc.sync.dma_start(
                            rs_in[qs + qc:qs + qc + cw, cs:cs + ccw],
                            pj_sb[:cw, :ccw])

        # ---- reduce-scatter over the 4 cores of this batch ---------------
        if with_rs == "split2" and (kp * 7 // 8) // 128 * 128 >= 128:
            # two chunked collectives: the first one's data phase overlaps
            # the second span's compute + projection
            c1 = (kp * 7 // 8) // 128 * 128   # = first span width (row count)
            c1 -= c1 % 4
            c2 = kpb - c1
            nc.gpsimd.collective_compute(
                "ReduceScatter", mybir.AluOpType.add,
                replica_groups=[[0, 1, 2, 3], [4, 5, 6, 7]],
                ins=[rs_in[0:c1]], outs=[rs_out[0:c1 // 4]])
            nc.gpsimd.collective_compute(
                "ReduceScatter", mybir.AluOpType.add,
                replica_groups=[[0, 1, 2, 3], [4, 5, 6, 7]],
                ins=[rs_in[c1:kpb]], outs=[rs_out[c1 // 4:kpb // 4]])
            nc.sync.dma_start(out_ext[:], rs_out[:])
        elif with_rs:
            nc.gpsimd.collective_compute(
                "ReduceScatter", mybir.AluOpType.add,
                replica_groups=[[0, 1, 2, 3], [4, 5, 6, 7]],
                ins=[rs_in[:]], outs=[rs_out[:]])
            nc.sync.dma_start(out_ext[:], rs_out[:])
        else:
            nc.sync.dma_start(out_ext[:], rs_in[0:kpb // 4, :])


def make_in_maps(inputs, kept, cnt, kp):
    x = np.asarray(inputs["x"], dtype=np.float32)
    Wq, Wk, Wv, Wo = (np.asarray(inputs[k], np.float32)
                      for k in ("Wq", "Wk", "Wv", "Wo"))
    bo = np.asarray(inputs["bo"], np.float32)
    woT_full = np.ascontiguousarray(Wo.T)          # [hd_in, c_out]
    in_maps = []
    for c in range(NCORES):
        b, g = divmod(c, NGROUPS)
        hs = slice(g * HD, (g + 1) * HD)
        xc = np.zeros((kp, C), np.float32)
        xc[:cnt[b]] = x[b][kept[b]]
        kcb_flat = np.full(kp, -1000.0, np.float32)
        kcb_flat[:cnt[b]] = 0.0
        kcb = np.ascontiguousarray(kcb_flat.reshape(kp // 128, 128).T)
        in_maps.append({
            "xcT": np.ascontiguousarray(xc.T).astype(NPBF16),
            "kcb": kcb,
            "wqT": np.ascontiguousarray(Wq[hs].T).astype(NPBF16),
            "wkT": np.ascontiguousarray(Wk[hs].T).astype(NPBF16),
            "wvT": np.ascontiguousarray(Wv[hs].T).astype(NPBF16),
            "woT": np.ascontiguousarray(woT_full[hs]).astype(NPBF16),
            "mxT": x[b].mean(0).reshape(C, 1).astype(NPBF16),
            "bo4": (bo / NGROUPS).reshape(1, C).astype(NPBF16),
        })
    return in_maps


RS_MODE = "split2"        # "split2" | True (single RS) — both in bf16


def kernel(x, mask, Wq, Wk, Wv, Wo, bo):
    x = np.asarray(x, dtype=np.float32)
    mask = np.asarray(mask)
    kept = [np.nonzero(mask[b])[0] for b in range(B)]
    cnt = [len(k) for k in kept]
    cnt_max = max(max(cnt), 1)
    kp = max(128, -(-cnt_max // 128) * 128)

    nc = _build(kp, cnt_max, with_rs=RS_MODE, rs_dt=BF16)
    in_maps = make_in_maps(
        {"x": x, "Wq": Wq, "Wk": Wk, "Wv": Wv, "Wo": Wo, "bo": bo},
        kept, cnt, kp)

    r = run_bass_kernel_spmd(nc, in_maps, core_ids=list(range(NCORES)))
    global LAST_HW_NS, LAST_RESULT
    LAST_RESULT = r
    if getattr(r, "exec_time_ns", None):
        LAST_HW_NS = r.exec_time_ns

    kpb = kp + 4
    out = np.empty((B, N, C), np.float32)
    for b in range(B):
        shards = [np.asarray(r.results[NGROUPS * b + i]["out"], np.float32)
                  for i in range(NGROUPS)]
        if RS_MODE == "split2" and (kp * 7 // 8) // 128 * 128 >= 128:
            c1 = (kp * 7 // 8) // 128 * 128
            c1 -= c1 % 4
            q1, q2 = c1 // 4, (kpb - c1) // 4
            rs = np.empty((kpb, C), np.float32)
            for i in range(NGROUPS):
                rs[q1 * i:q1 * i + q1] = shards[i][:q1]
                rs[c1 + q2 * i:c1 + q2 * i + q2] = shards[i][q1:]
        else:
            rs = np.concatenate(shards, axis=0)
        out[b, kept[b]] = rs[:cnt[b]]
        out[b, mask[b] == 0] = rs[kp]
    return out

